# revision 104
# baseline (speedup 1.0000x reference)
"""Trainium2 Bass kernel for nn_LowRankSVDBlock (dense transformer block with
low-rank SVD projections), tensor-parallel over 8 NeuronCores.

Sharding:
  Phase 1 (attention): tensor-parallel over heads — core c computes heads
  {2c, 2c+1} for both batches: LN1 (replicated stats via tiny AllGather),
  fp8 DoubleRow low-rank latent projections, causal attention in bilinear
  A-form (scores = P_k^T M P_q, skipping Q/K reconstruction), producing
  ctx^T (fp8, x16) for its 2 heads x all tokens. Two AllToAlls (one per
  batch) redistribute ctx from head-sharded to token-sharded layout.
  Phase 2 (out-proj + MLP): token-parallel — core c handles 512 tokens
  (256 from each batch): out_U/out_V projection, residual, LN2, low-rank MLP.
  The phase-2 front (out-proj/LN2/fc1_U) runs per-batch so batch 0's front
  overlaps batch 1's AllToAll.

Heavy GEMMs run as fp8e4m3 DoubleRow matmuls (K=256 per instruction, half
cycles per output row). Weights are pre-scaled by 2^9 and activations by
small powers of two on the fp8 path; descaling folds into existing PSUM
evacuation ops.
"""
import sys

import ml_dtypes
import numpy as np

sys.path.insert(0, "/opt/trn_rl_repo")

import concourse.bass as bass  # noqa: E402,F401
import concourse.tile as tile  # noqa: E402
from concourse import bacc, mybir  # noqa: E402
from concourse.bass_utils import run_bass_kernel_spmd  # noqa: E402

F32 = mybir.dt.float32
F32R = mybir.dt.float32r
BF16 = mybir.dt.bfloat16
F8 = mybir.dt.float8e4
AX = mybir.AluOpType
AF = mybir.ActivationFunctionType
DR = mybir.MatmulPerfMode.DoubleRow

NC = 8
B, S, D, H = 2, 2048, 1024, 16
DH, R, ROUT, INNER, RMLP = 64, 48, 768, 4096, 512
T = B * S          # 4096 flat tokens
TSH = T // NC      # 512 tokens per core in phase 2
HSH = TSH // 2     # 256 tokens per batch per core
LN_EPS = 1e-5
SC13 = 2.0 ** 13   # fp8 weight (2^9) x activation (2^4) scale product

_NC_CACHE = {}


def _build():
    nc = bacc.Bacc()

    # ---- external inputs (per-core, host-prepped) ----
    hidt_e = nc.dram_tensor("hidt", [128, 8, T], F8, kind="ExternalInput")
    negcsg_e = nc.dram_tensor("negcsg", [2, 384], F32, kind="ExternalInput")
    hidsh_e = nc.dram_tensor("hidsh", [TSH, D], BF16, kind="ExternalInput")
    hidshb_e = nc.dram_tensor("hidshb", [TSH, D], BF16, kind="ExternalInput")
    wu_e = nc.dram_tensor("wu", [128, 8, 384], F8, kind="ExternalInput")
    m2_e = nc.dram_tensor("m2", [128, 128], BF16, kind="ExternalInput")
    wv2v_e = nc.dram_tensor("wv2v", [128, 128], BF16, kind="ExternalInput")
    wout_e = nc.dram_tensor("wout", [128, 8, ROUT], F8, kind="ExternalInput")
    wov_e = nc.dram_tensor("wov", [128, 6, D], F8, kind="ExternalInput")
    wf1_e = nc.dram_tensor("wf1", [128, 8, RMLP], F8, kind="ExternalInput")
    # per it-pair: fc1_V block [k(4), u(2), c(128)] ++ fc2_U block [u(2), m(512)]
    wmid_e = nc.dram_tensor("wmid", [16, 128, 2048], F8, kind="ExternalInput")
    wf2v_e = nc.dram_tensor("wf2v", [8, 128, 4, 128], F8, kind="ExternalInput")
    cb1_e = nc.dram_tensor("cb1", [1, RMLP], F32, kind="ExternalInput")
    f1b_e = nc.dram_tensor("f1b", [128, 32], F32, kind="ExternalInput")
    f2b_e = nc.dram_tensor("f2b", [128, 8], F32, kind="ExternalInput")
    masks2_e = nc.dram_tensor("masks2", [128, 4, 2, 512], BF16, kind="ExternalInput")
    ones_e = nc.dram_tensor("ones", [1, 512], F32, kind="ExternalInput")
    eyeb_e = nc.dram_tensor("eyeb", [128, 128], BF16, kind="ExternalInput")
    ones16_e = nc.dram_tensor("ones16", [1, 512], BF16, kind="ExternalInput")

    out_e = nc.dram_tensor("out", [TSH, D], BF16, kind="ExternalOutput")

    # internal DRAM for the collectives
    ag_in = nc.dram_tensor("ag_in", [1, 2048], F32)
    ag_out = nc.dram_tensor("ag_out", [NC, 2048], F32, addr_space="Shared")
    a2a_in = [nc.dram_tensor(f"a2a_in{b}", [NC * 128, HSH], F8) for b in range(B)]
    a2a_out = [nc.dram_tensor(f"a2a_out{b}", [NC * 128, HSH], F8) for b in range(B)]
    rgroups = [list(range(NC))]

    with tile.TileContext(nc) as tc, nc.allow_low_precision(reason="f32r/fp8 matmul tags"):
        with tc.tile_pool(name="consts", bufs=1) as cp:
            identb = cp.tile([128, 128], BF16, tag="identb")
            nc.sync.dma_start(out=identb, in_=eyeb_e[:, :])
            eps_t = cp.tile([128, 1], F32, tag="eps")
            nc.vector.memset(eps_t, LN_EPS)
            eps26_t = cp.tile([128, 1], F32, tag="eps26")
            nc.vector.memset(eps26_t, LN_EPS * SC13 * SC13)
            ones_t = cp.tile([1, 512], F32R, tag="ones")
            nc.sync.dma_start(out=ones_t, in_=ones_e[:, :].bitcast(F32R))
            masks2_t = cp.tile([128, 4, 2, 512], BF16, tag="masks2")
            cb1_t = cp.tile([1, RMLP], F32R, tag="cb1")
            f1b_t = cp.tile([128, 32], F32, tag="f1b")
            f2b_t = cp.tile([128, 8], F32, tag="f2b")
            # phase-2 weights (DMA'd early; they overlap phase-1 compute)
            wout_t = cp.tile([128, 8, ROUT], F8, tag="wout")
            wov_t = cp.tile([128, 6, D], F8, tag="wov")
            wf1_t = cp.tile([128, 8, RMLP], F8, tag="wf1")

            _phase1(nc, tc, hidsh_e, hidt_e, negcsg_e, ag_in, ag_out, wu_e,
                    m2_e, wv2v_e, ones_e, ones16_e, masks2_e, masks2_t,
                    ones_t, eps_t, eps26_t, a2a_in, a2a_out, rgroups,
                    [(wout_t, wout_e), (wov_t, wov_e), (wf1_t, wf1_e)],
                    [(cb1_t, cb1_e, F32R), (f1b_t, f1b_e, None),
                     (f2b_t, f2b_e, None)])
            _phase2(nc, tc, a2a_out, hidshb_e, wout_t, wov_t, wf1_t, wmid_e,
                    wf2v_e, cb1_t, f1b_t, f2b_t, eps_t, identb,
                    ones_t, out_e)

    nc.finalize()
    return nc


def _phase1(nc, tc, hidsh_e, hidt_e, negcsg_e, ag_in, ag_out, wu_e, m2_e,
            wv2v_e, ones_e, ones16_e, masks2_e, masks2_t, ones_t, eps_t,
            eps26_t, a2a_in, a2a_out, rgroups, p2_weights, p2_consts):
    """Head-sharded: LN1, fp8-DR latent projections, A-form attention, A2A."""
    with tc.tile_pool(name="p1big", bufs=1) as bigp:
        # latent projections P~ = rstd (.) (Ug^T x - CSg mu); rows:
        # h0 latents 0:48, ones row 48, h1 latents 64:112, ones row 112
        # (rows 48/112 become exactly 1.0 via the std13 trick).
        pbuf = [bigp.tile([128, T], BF16, tag=f"P{i}", name=f"P{i}") for i in range(3)]
        a_buf = bigp.tile([128, T], BF16, tag="AB")
        # V natural [tok, dh]+ones col, per (b, h): [:, b*2+h, kt, :]
        vn_buf = bigp.tile([128, 4, 16, 65], BF16, tag="VN")
        wu_t = bigp.tile([128, 8, 384], F8, tag="wu")
        m2_t = bigp.tile([128, 128], BF16, tag="m2")
        wv2v_t = bigp.tile([128, 128], BF16, tag="wv2v")
        mustd_t = bigp.tile([2, T], F32R, tag="mustd")
        r13_t = bigp.tile([1, T], F32R, tag="r13")

        # ---------- stage A+B: sharded LN1 stats + AllGather + folded-LN
        # fp8-DR U-projections.
        with tc.tile_pool(name="pA", bufs=4) as ap_, \
             tc.tile_pool(name="pAs", bufs=8) as sp_, \
             tc.tile_pool(name="pAx", bufs=5) as xp_, \
             tc.tile_pool(name="pAr", bufs=4) as rp_, \
             tc.tile_pool(name="psB", bufs=6, space="PSUM") as psB, \
             tc.tile_pool(name="psR", bufs=2, space="PSUM") as psR:
            # local LN1 stats on this core's 512 tokens; ship mu, var,
            # 2^13*std, and rstd*2^-13 so downstream scaling folds into the
            # gather. All input DMAs are emitted before the (stat-dependent)
            # gather-staging DMAs so the DMA queue never stalls early.
            nats, stts = [], []
            for tl in range(4):
                nat = ap_.tile([128, D], BF16, tag="nat")
                nc.sync.dma_start(out=nat, in_=hidsh_e[tl * 128:(tl + 1) * 128, :])
                nats.append(nat)
            hidt_tiles = {}
            for bb in range(3):
                ht = xp_.tile([128, 8, 512], F8, tag="hidt", name=f"hidt{bb}")
                nc.sync.dma_start(out=ht,
                                  in_=hidt_e[:, :, bb * 512:(bb + 1) * 512])
                hidt_tiles[bb] = ht
            for tl in range(4):
                nat = nats[tl]
                st = sp_.tile([128, 2, 6], F32, tag="st")
                nc.vector.bn_stats(out=st[:, 0, :], in_=nat[:, 0:512])
                nc.vector.bn_stats(out=st[:, 1, :], in_=nat[:, 512:1024])
                stt = sp_.tile([128, 4], F32, tag="stt")
                nc.vector.bn_aggr(out=stt[:, 0:2], in_=st)
                nc.scalar.activation(out=stt[:, 2:3], in_=stt[:, 1:2], func=AF.Sqrt,
                                     bias=eps26_t[:, :], scale=SC13 * SC13)
                nc.vector.reciprocal(stt[:, 3:4], stt[:, 2:3])
                stts.append(stt)
            for tl in range(4):
                nc.sync.dma_start(
                    out=ag_in[0:1, :].rearrange("o (w t) -> o t w", w=4)
                    [:, tl * 128:(tl + 1) * 128, :],
                    in_=stts[tl][:, :])
            nc.gpsimd.collective_compute(
                "AllGather", AX.bypass, ins=[ag_in[:, :]], outs=[ag_out[:, :]],
                replica_groups=rgroups)
            # weight / const loads (overlap the stats+gather)
            nc.sync.dma_start(out=wu_t, in_=wu_e[:, :, :])
            nc.sync.dma_start(out=m2_t, in_=m2_e[:, :])
            nc.sync.dma_start(out=wv2v_t, in_=wv2v_e[:, :])
            negcsg_t = bigp.tile([2, 384], F32R, tag="negcsg")
            nc.sync.dma_start(out=negcsg_t, in_=negcsg_e[:, :].bitcast(F32R))
            for bh in range(4):
                nc.sync.dma_start(
                    out=vn_buf[:, bh, :, 64:65],
                    in_=ones16_e[0:1, 0:1].to_broadcast([128, 16, 1]))
            nc.sync.dma_start(out=masks2_t, in_=masks2_e[:, :, :, :])
            for wt, we in p2_weights:
                nc.sync.dma_start(out=wt, in_=we[:, :, :])
            for ct, ce, cast in p2_consts:
                if cast is None:
                    nc.sync.dma_start(out=ct, in_=ce[:, :])
                else:
                    nc.sync.dma_start(out=ct, in_=ce[:, :].bitcast(cast))
            # consolidated per-token stat rows: block bb covers flat tokens
            # [bb*512,(bb+1)*512) = cores (2*(bb%4), 2*(bb%4)+1), half bb//4.
            # ag_out[(bm hf), (which, hb, i)] -> flat (hb bm hf i) token order.
            for which, row in ((0, 0), (2, 1)):
                nc.sync.dma_start(
                    out=mustd_t[row:row + 1, :].rearrange(
                        "o (hb bm hf i) -> o hb bm hf i", hb=2, bm=4, hf=2),
                    in_=ag_out[0:8, which * 512:(which + 1) * 512]
                    .rearrange("(bm hf) (hb i) -> hb bm hf i", hf=2, hb=2)
                    .bitcast(F32R))
            nc.sync.dma_start(
                out=r13_t[0:1, :].rearrange(
                    "o (hb bm hf i) -> o hb bm hf i", hb=2, bm=4, hf=2),
                in_=ag_out[0:8, 1536:2048]
                .rearrange("(bm hf) (hb i) -> hb bm hf i", hf=2, hb=2)
                .bitcast(F32R))

            for bb in range(8):          # 512-token blocks
                if bb in hidt_tiles:
                    hidt_t = hidt_tiles[bb]
                else:
                    hidt_t = xp_.tile([128, 8, 512], F8, tag="hidt")
                    nc.sync.dma_start(out=hidt_t,
                                      in_=hidt_e[:, :, bb * 512:(bb + 1) * 512])
                cols = slice(bb * 512, (bb + 1) * 512)
                psr = psR.tile([128, 512], F32, tag="ps_r")
                nc.tensor.matmul(psr[:, :], ones_t[0:1, 0:128],
                                 r13_t[0:1, cols], start=True, stop=True)
                rstdb = rp_.tile([128, 512], F32, tag="rstdb")
                nc.scalar.copy(out=rstdb, in_=psr)
                # DR U-projections + fused (mu-correction, 2^13*std ones-row)
                for pi in range(3):
                    psu = psB.tile([128, 512], F32, tag="ps_u")
                    for j in range(4):
                        nc.tensor.matmul(psu[:, :],
                                         wu_t[:, 2 * j:2 * j + 2, pi * 128:(pi + 1) * 128],
                                         hidt_t[:, 2 * j:2 * j + 2, :],
                                         start=(j == 0), stop=False, perf_mode=DR)
                    nc.tensor.matmul(psu[:, :], negcsg_t[0:2, pi * 128:(pi + 1) * 128],
                                     mustd_t[0:2, cols], start=False, stop=True)
                    nc.vector.tensor_tensor(out=pbuf[pi][0:113, cols],
                                            in0=psu[0:113, :], in1=rstdb[0:113, :],
                                            op=AX.mult)

        # ---------- stage C: bilinear-form A = M2^T @ P_q, block-diag V ----
        with tc.tile_pool(name="psC", bufs=4, space="PSUM") as psC:
            for nt in range(8):
                ps = psC.tile([128, 512], F32, tag="ps_a")
                nc.tensor.matmul(ps[0:113, :], m2_t[0:113, 0:113],
                                 pbuf[0][0:113, nt * 512:(nt + 1) * 512],
                                 start=True, stop=True)
                nc.vector.tensor_copy(out=a_buf[0:113, nt * 512:(nt + 1) * 512],
                                      in_=ps[0:113, :])
            for b in range(B):
                for kt in range(16):
                    c0 = b * S + kt * 128
                    ps = psC.tile([128, 128], F32, tag="ps_v")
                    nc.tensor.matmul(ps[:, :], pbuf[2][0:113, c0:c0 + 128],
                                     wv2v_t[0:113, :], start=True, stop=True)
                    nc.scalar.copy(
                        out=vn_buf[:, 2 * b:2 * b + 2, kt, 0:64],
                        in_=ps[:, :].rearrange("p (a b) -> p a b", a=2))

        # ---------- stage D: causal attention per (batch, head-pair) + A2A --
        with tc.tile_pool(name="probs", bufs=18) as prp, \
             tc.tile_pool(name="ctxp", bufs=3) as ctp, \
             tc.tile_pool(name="psS", bufs=2, space="PSUM") as psS, \
             tc.tile_pool(name="psbp", bufs=1, space="PSUM") as psbp, \
             tc.tile_pool(name="psA2", bufs=3, space="PSUM") as psA2:
            for b in range(B):
                for qt in range(4):
                    nk = 4 * (qt + 1)
                    q0 = b * S + qt * 512
                    prs = {}
                    kt_order = list(range(nk))
                    for kt in kt_order:
                        j = kt - 4 * qt
                        # diagonal tile j: columns < j*128 are fully masked --
                        # restrict scores/exp/mask/ctx to the valid range.
                        v0 = max(j, 0) * 128
                        pss = psS.tile([128, 2, 512], F32, tag="ps_s")
                        for h in range(2):
                            nc.tensor.matmul(
                                pss[:, h, v0:512],
                                pbuf[1][h * 64:h * 64 + 49,
                                        b * S + kt * 128:b * S + (kt + 1) * 128],
                                a_buf[h * 64:h * 64 + 49, q0 + v0:q0 + 512],
                                start=True, stop=True)
                        pr = prp.tile([128, 2, 512], BF16, tag="pr")
                        nc.scalar.activation(out=pr[:, :, v0:512],
                                             in_=pss[:, :, v0:512],
                                             func=AF.Exp, scale=1.0)
                        if j >= 0:
                            nc.vector.tensor_tensor(
                                out=pr[:, :, v0:512], in0=pr[:, :, v0:512],
                                in1=masks2_t[:, j, :, v0:512], op=AX.mult)
                        prs[kt] = (pr, v0)
                    ctx_order = kt_order
                    for h in range(2):
                        psc = psA2.tile([65, 512], F32, tag="ps_c")
                        for i, kt in enumerate(ctx_order):
                            pr, v0 = prs[kt]
                            nc.tensor.matmul(psc[:, v0:512],
                                             vn_buf[:, b * 2 + h, kt, :],
                                             pr[:, h, v0:512], start=(i == 0),
                                             stop=(i == nk - 1))
                        # vn's extra column holds 1/16, so psc[64] = denom/16
                        # and rc = 16/denom -> ctx lands in fp8 at x16 scale.
                        rc = ctp.tile([1, 512], F32R, tag="rc")
                        nc.vector.reciprocal(rc, psc[64:65, :])
                        ctxu = ctp.tile([64, 512], BF16, tag="ctxu")
                        nc.vector.tensor_copy(ctxu, psc[0:64, :])
                        psb = psbp.tile([64, 512], F32, tag="ps_b")
                        nc.tensor.matmul(psb[:, :], ones_t[0:1, 0:64], rc,
                                         start=True, stop=True)
                        ctx = ctp.tile([64, 512], F8, tag="ctx")
                        nc.vector.tensor_tensor(out=ctx, in0=ctxu, in1=psb,
                                                op=AX.mult)
                        # both half-shards in one DMA: out rows iterate
                        # (r, hf) to match ctx's (partition, hf-block) order
                        nc.sync.dma_start(
                            out=a2a_in[b][2 * qt * 128:(2 * qt + 2) * 128, :]
                            .rearrange("(hf rr) n -> rr hf n", hf=2)
                            [h * 64:(h + 1) * 64, :, :],
                            in_=ctx[:, :].rearrange("p (hf n) -> p hf n", hf=2))
                # launch this batch's A2A as soon as its ctx is written
                nc.gpsimd.collective_compute(
                    "AllToAll", AX.bypass, ins=[a2a_in[b][:, :]],
                    outs=[a2a_out[b][:, :]], replica_groups=rgroups)


def _phase2(nc, tc, a2a_out, hidshb_e, wout_t, wov_t, wf1_t, wmid_e,
            wf2v_e, cb1_t, f1b_t, f2b_t, eps_t, identb, ones_t, out_e):
    """Token-sharded: out-projection, residual, LN2, low-rank MLP, output.

    The front (out-proj + LN2 + fc1_U) runs per batch: batch 0's front only
    depends on A2A#0 so it overlaps A2A#1.
    """
    with tc.tile_pool(name="p2big", bufs=1) as bigp, \
         tc.tile_pool(name="p2st", bufs=4) as sp_, \
         tc.tile_pool(name="mstr", bufs=6) as msp:
        hnat = bigp.tile([128, 4, D], BF16, tag="hnat")
        x2T = bigp.tile([128, 8, TSH], F8, tag="x2T")
        t1T = bigp.tile([128, 4, TSH], F8, tag="t1T")
        poT = bigp.tile([128, 6, TSH], F8, tag="poT")
        t2T = bigp.tile([128, 4, TSH], F8, tag="t2T")
        outsb = [bigp.tile([128, D], BF16, tag=f"osb{q}", name=f"osb{q}")
                 for q in range(4)]

        def _front(b, pa, psFM, psTrF):
            """out-proj + residual + LN2 + fc1_U for batch b's token half."""
            bs = slice(b * HSH, (b + 1) * HSH)
            ctxT = pa.tile([128, 8, HSH], F8, tag="ctxT")
            nc.sync.dma_start(
                out=ctxT,
                in_=a2a_out[b][:, :].rearrange("(j p) n -> p j n", p=128))
            for ro in range(6):
                ps = psFM.tile([128, HSH], F32, tag="ps_f")
                for j in range(4):
                    nc.tensor.matmul(
                        ps[:, :], wout_t[:, 2 * j:2 * j + 2, ro * 128:(ro + 1) * 128],
                        ctxT[:, 2 * j:2 * j + 2, :], start=(j == 0), stop=(j == 3),
                        perf_mode=DR)
                nc.scalar.activation(out=poT[:, ro, bs], in_=ps,
                                     func=AF.Copy, scale=2.0 ** -7)
            for tt in (2 * b, 2 * b + 1):
                hs = sp_.tile([128, D], BF16, tag="hs")
                nc.sync.dma_start(out=hs, in_=hidshb_e[tt * 128:(tt + 1) * 128, :])
                for nq in range(4):
                    ps = psFM.tile([128, HSH], F32, tag="ps_f")
                    for j in range(3):
                        nc.tensor.matmul(
                            ps[:, :], poT[:, 2 * j:2 * j + 2, tt * 128:(tt + 1) * 128],
                            wov_t[:, 2 * j:2 * j + 2, nq * 256:(nq + 1) * 256],
                            start=(j == 0), stop=(j == 2), perf_mode=DR)
                    nc.vector.scalar_tensor_tensor(
                        out=hnat[:, tt, nq * 256:(nq + 1) * 256], in0=ps,
                        scalar=2.0 ** -15, in1=hs[:, nq * 256:(nq + 1) * 256],
                        op0=AX.mult, op1=AX.add)
                st = sp_.tile([128, 2, 6], F32, tag="st2")
                nc.vector.bn_stats(out=st[:, 0, :], in_=hnat[:, tt, 0:512])
                nc.vector.bn_stats(out=st[:, 1, :], in_=hnat[:, tt, 512:1024])
                mv = sp_.tile([128, 2], F32, tag="mv2")
                nc.vector.bn_aggr(out=mv, in_=st)
                rstd = sp_.tile([128, 1], F32, tag="rstd2")
                nc.scalar.activation(out=rstd, in_=mv[:, 1:2], func=AF.Sqrt,
                                     bias=eps_t[:, :], scale=1.0)
                nc.vector.reciprocal(rstd, rstd)
                xh = sp_.tile([128, D], BF16, tag="xh2")
                nc.vector.tensor_scalar(out=xh, in0=hnat[:, tt, :], scalar1=mv[:, 0:1],
                                        scalar2=rstd, op0=AX.subtract, op1=AX.mult)
                for g in range(2):
                    pst = psTrF.tile([128, 512], BF16, tag="ps_tr")
                    for q in range(4):
                        nc.tensor.transpose(pst[:, q * 128:(q + 1) * 128],
                                            xh[:, (4 * g + q) * 128:(4 * g + q + 1) * 128],
                                            identb)
                    nc.scalar.activation(
                        out=x2T[:, 4 * g:4 * g + 4, tt * 128:(tt + 1) * 128],
                        in_=pst, func=AF.Copy, scale=16.0)
            # t1^T for this batch half
            for m in range(4):
                ps = psFM.tile([128, HSH], F32, tag="ps_f")
                for j in range(4):
                    nc.tensor.matmul(ps[:, :],
                                     wf1_t[:, 2 * j:2 * j + 2, m * 128:(m + 1) * 128],
                                     x2T[:, 2 * j:2 * j + 2, bs],
                                     start=(j == 0), stop=False, perf_mode=DR)
                nc.tensor.matmul(ps[:, :], cb1_t[0:1, m * 128:(m + 1) * 128],
                                 ones_t[0:1, 0:HSH], start=False, stop=True)
                nc.scalar.activation(out=t1T[:, m, bs], in_=ps,
                                     func=AF.Copy, scale=2.0 ** -13)

        # fronts (front0 fills the A2A#1 wait), then the full-width mid-MLP
        with tc.tile_pool(name="p2a", bufs=2) as pa, \
             tc.tile_pool(name="psFM", bufs=4, space="PSUM") as psFM:
            _front(0, pa, psFM, psFM)
            _front(1, pa, psFM, psFM)

        # ---- fused mid-MLP (fc1_V -> gelu -> fc2_U), it-pairs for DR ----
        with tc.tile_pool(name="psM", bufs=3, space="PSUM") as psM, \
             tc.tile_pool(name="psT2", bufs=1, space="PSUM") as psT2:
            t2ps = [psT2.tile([128, TSH], F32, tag=f"ps_t2_{rt}", name=f"ps_t2_{rt}")
                    for rt in range(4)]
            for t in range(16):
                wmid = msp.tile([128, 2048], F8, tag="wmid")
                nc.sync.dma_start(out=wmid, in_=wmid_e[t, :, :])
                f1v = wmid[:, 0:1024].rearrange("p (k u c) -> p k u c", k=4, u=2)
                f2u = wmid[:, 1024:2048].rearrange("p (u m) -> p u m", u=2)
                mt = msp.tile([128, 2, TSH], F8, tag="mt")
                for u in range(2):
                    psm = psM.tile([128, TSH], F32, tag="ps_m")
                    for kp in range(2):
                        nc.tensor.matmul(psm[:, :], f1v[:, 2 * kp:2 * kp + 2, u, :],
                                         t1T[:, 2 * kp:2 * kp + 2, :],
                                         start=(kp == 0), stop=(kp == 1), perf_mode=DR)
                    it = 2 * t + u
                    nc.scalar.activation(out=mt[:, u, :], in_=psm,
                                         func=AF.Gelu_apprx_tanh,
                                         bias=f1b_t[:, it:it + 1], scale=2.0 ** -9)
                for rt in range(4):
                    nc.tensor.matmul(t2ps[rt][:, :], f2u[:, :, rt * 128:(rt + 1) * 128],
                                     mt[:, :, :], start=(t == 0), stop=(t == 15),
                                     perf_mode=DR)
            for rt in range(4):
                nc.vector.tensor_scalar_mul(out=t2T[:, rt, :], in0=t2ps[rt],
                                            scalar1=2.0 ** -5)

        # ---- mlp^T -> +fc2_b -> transpose -> + h_nat -> out ----
        with tc.tile_pool(name="psE", bufs=4, space="PSUM") as psE, \
             tc.tile_pool(name="psTrE", bufs=1, space="PSUM") as psTrE:
            for g in range(2):
                ptr = [psTrE.tile([128, 512], BF16, tag=f"ps_tr3_{q}",
                                  name=f"ps_tr3_{g}_{q}") for q in range(4)]
                for di in range(4):
                    dt_ = 4 * g + di
                    f2v = msp.tile([128, 4, 128], F8, tag="f2v")
                    nc.sync.dma_start(out=f2v, in_=wf2v_e[dt_, :, :, :])
                    ps = psE.tile([128, TSH], F32, tag="ps_e")
                    for kp in range(2):
                        nc.tensor.matmul(ps[:, :], f2v[:, 2 * kp:2 * kp + 2, :],
                                         t2T[:, 2 * kp:2 * kp + 2, :],
                                         start=(kp == 0), stop=(kp == 1), perf_mode=DR)
                    mo = sp_.tile([128, TSH], BF16, tag="mo")
                    nc.scalar.activation(out=mo, in_=ps, func=AF.Identity,
                                         bias=f2b_t[:, dt_:dt_ + 1],
                                         scale=2.0 ** -13)
                    for q4 in range(4):
                        nc.tensor.transpose(ptr[q4][:, di * 128:(di + 1) * 128],
                                            mo[:, q4 * 128:(q4 + 1) * 128], identb)
                for q4 in range(4):
                    nc.vector.tensor_tensor(
                        out=outsb[q4][:, g * 512:(g + 1) * 512],
                        in0=hnat[:, q4, g * 512:(g + 1) * 512], in1=ptr[q4], op=AX.add)
                    nc.sync.dma_start(
                        out=out_e[q4 * 128:(q4 + 1) * 128, g * 512:(g + 1) * 512],
                        in_=outsb[q4][:, g * 512:(g + 1) * 512])


def _prep_inputs(inputs):
    """Host-side sharding/packing of inputs into per-core in_maps."""
    f = np.float32
    FP8 = ml_dtypes.float8_e4m3
    S9 = np.float32(2.0 ** 9)
    S4 = np.float32(2.0 ** 4)
    hid = np.ascontiguousarray(np.asarray(inputs["hidden_states"]).reshape(T, D)).astype(f)
    ln1_g = np.asarray(inputs["ln1_g"], f)
    ln1_b = np.asarray(inputs["ln1_b"], f)
    ln2_g = np.asarray(inputs["ln2_g"], f)
    ln2_b = np.asarray(inputs["ln2_b"], f)
    out_b = np.asarray(inputs["out_b"], f)
    scale = np.float32(1.0 / np.sqrt(DH))

    wout = np.ascontiguousarray(
        (np.asarray(inputs["out_U"], f) * S9).reshape(8, 128, ROUT)
        .transpose(1, 0, 2).astype(FP8))
    wov = np.ascontiguousarray(
        (np.asarray(inputs["out_V"], f) * S9).reshape(6, 128, D)
        .transpose(1, 0, 2).astype(FP8))
    fc1U = np.asarray(inputs["fc1_U"], f)
    wf1 = np.ascontiguousarray(
        ((fc1U * ln2_g[:, None]) * S9).reshape(8, 128, RMLP)
        .transpose(1, 0, 2).astype(FP8))
    cb1 = np.ascontiguousarray(((ln2_b @ fc1U) * 2.0 ** 13).reshape(1, RMLP))
    wf1v = ((np.asarray(inputs["fc1_V"], f) * S9).reshape(4, 128, 16, 2, 128)
            .transpose(2, 1, 0, 3, 4))          # [t, p, k, u, c]
    wf2u = ((np.asarray(inputs["fc2_U"], f) * S9).reshape(16, 2, 128, RMLP)
            .transpose(0, 2, 1, 3))             # [t, p, u, m]
    wmid = np.ascontiguousarray(np.concatenate(
        [wf1v.reshape(16, 128, 1024), wf2u.reshape(16, 128, 1024)],
        axis=2).astype(FP8))
    wf2v = np.ascontiguousarray(
        (np.asarray(inputs["fc2_V"], f) * S9).reshape(4, 128, 8, 128)
        .transpose(2, 1, 0, 3).astype(FP8))
    f1b = np.ascontiguousarray(np.asarray(inputs["fc1_b"], f).reshape(32, 128).T)
    f2b = np.ascontiguousarray(np.asarray(inputs["fc2_b"], f).reshape(8, 128).T)
    hidt = np.ascontiguousarray(
        (hid * S4).reshape(T, 8, 128).transpose(2, 1, 0).astype(FP8))
    masks2 = np.zeros((128, 4, 2, 512), f)
    for j in range(4):
        valid = (np.arange(128)[:, None] <= np.arange(512)[None, :] - 128 * j).astype(f)
        masks2[:, j, 0, :] = valid
        masks2[:, j, 1, :] = valid
    masks2 = masks2.astype(ml_dtypes.bfloat16)
    ones = np.ones((1, 512), f)
    # vn's denominator column: 1/16 so the softmax reciprocal comes out
    # pre-scaled by 16 (the fp8 ctx scale)
    ones16 = np.full((1, 512), 1.0 / 16.0, ml_dtypes.bfloat16)
    eyeb = np.eye(128, dtype=ml_dtypes.bfloat16)

    qU = np.asarray(inputs["q_U"], f)
    kU = np.asarray(inputs["k_U"], f)
    vU = np.asarray(inputs["v_U"], f)
    qV = np.asarray(inputs["q_V"], f)
    kV = np.asarray(inputs["k_V"], f)
    vV = np.asarray(inputs["v_V"], f)
    qb = np.asarray(inputs["q_b"], f)
    kb = np.asarray(inputs["k_b"], f)
    vb = np.asarray(inputs["v_b"], f)

    in_maps = []
    for c in range(NC):
        h0 = 2 * c
        wu = np.zeros((D, 3, 128), f)
        for pi, u in enumerate((qU, kU, vU)):
            wu[:, pi, 0:48] = u[:, h0, :] * ln1_g[:, None]
            wu[:, pi, 64:112] = u[:, h0 + 1, :] * ln1_g[:, None]
        wu = wu.reshape(8, 128, 3, 128).transpose(1, 0, 2, 3).reshape(128, 8, 384)
        wu = np.ascontiguousarray((wu * S9).astype(FP8))
        # row 0: mu-correction weights (fp8-exact sums x hidt scale 2^4);
        # row 1: indicator that injects 2^13*std into the ones rows 48/112.
        negcsg2 = np.zeros((2, 384), f)
        negcsg2[0] = -wu.astype(np.float32).sum(axis=(0, 1)) * S4
        for pi in range(3):
            negcsg2[1, pi * 128 + 48] = 1.0
            negcsg2[1, pi * 128 + 112] = 1.0
        # bilinear attention form M2 (per head: scale * [Vq;qbeta] @ [Vk;kbeta]^T)
        m2 = np.zeros((128, 128), f)
        wv2v = np.zeros((128, 128), f)
        for hh in range(2):
            h = h0 + hh
            base = hh * 64
            qbeta = (ln1_b @ qU[:, h, :]) @ qV[h] + qb[h]
            kbeta = (ln1_b @ kU[:, h, :]) @ kV[h] + kb[h]
            wq_aug = np.concatenate([qV[h], qbeta[None, :]], axis=0)  # [49, 64]
            wk_aug = np.concatenate([kV[h], kbeta[None, :]], axis=0)  # [49, 64]
            m2[base:base + 49, base:base + 49] = scale * (wq_aug @ wk_aug.T)
            vbeta = (ln1_b @ vU[:, h, :]) @ vV[h] + vb[h]
            wv2v[base:base + 48, base:base + 64] = vV[h]
            wv2v[base + 48, base:base + 64] = vbeta
        m2 = m2.astype(ml_dtypes.bfloat16)
        wv2v = wv2v.astype(ml_dtypes.bfloat16)
        hidsh = np.ascontiguousarray(
            np.concatenate([hid[c * HSH:(c + 1) * HSH],
                            hid[S + c * HSH:S + (c + 1) * HSH]],
                           axis=0).astype(ml_dtypes.bfloat16))
        hidshb = np.ascontiguousarray(
            (hidsh.astype(np.float32) + out_b[None, :]).astype(ml_dtypes.bfloat16))
        in_maps.append({
            "hidt": hidt, "negcsg": negcsg2,
            "hidsh": hidsh, "hidshb": hidshb, "wu": wu, "m2": m2,
            "wv2v": wv2v,
            "wout": wout, "wov": wov, "wf1": wf1, "wmid": wmid,
            "wf2v": wf2v, "cb1": cb1, "f1b": f1b, "f2b": f2b,
            "masks2": masks2, "ones": ones, "ones16": ones16,
            "eyeb": eyeb,
        })
    return in_maps


def _assemble(results):
    out = np.empty((T, D), np.float32)
    for c in range(NC):
        r = np.asarray(results[c]["out"], dtype=np.float32)
        out[c * HSH:(c + 1) * HSH] = r[:HSH]
        out[S + c * HSH:S + (c + 1) * HSH] = r[HSH:]
    return out.reshape(B, S, D)


def kernel(**inputs):
    if "nc" not in _NC_CACHE:
        _NC_CACHE["nc"] = _build()
    nc = _NC_CACHE["nc"]
    in_maps = _prep_inputs(inputs)
    res = run_bass_kernel_spmd(nc, in_maps, list(range(NC)))
    return _assemble(res.results)


if __name__ == "__main__":
    print("kernel module ok")
